# revision 6
# baseline (speedup 1.0000x reference)
"""Single-head attention (B=4, S=4096, D=1024, DK=DV=128) on 8 TRN2 NeuronCores.

Sharding: key-parallel -> core i handles batch i//2, KEY rows
[h*2048, (h+1)*2048) with h = i%2, for ALL 4096 queries; emits unnormalized
partial numerator O^T and exp-sum partials; host combines + normalizes.
Host prep: bf16 cast, fold 1/sqrt(DK) into Wq/bq, swizzle to 8KB lines;
bk dropped (cancels in softmax); bv added on host.

v3 schedule.  The kernel is bound by the ACT engine's exp stream (64 tiles x
~1.11us) and the input wire (~16.8MB at ~310 GB/s); everything else is
arranged so neither ever waits:

- 2x2 phase blocking: P1=blocks01 x chunks01, P2=blocks23 x chunks01,
  P3=blocks01 x chunks23, P4=blocks23 x chunks23.  O^T partials spill
  PSUM->SBUF bf16 after P1/P3 passes and merge in P2/P4.  This spreads DMA
  deadlines to match the single need-order wire stream (q2q3 by tile 8,
  kv2/3 by tiles 16-21, q4-7 by tile 32-40).
- PV skew: each tile's PV matmuls are emitted one tile late (a pending
  queue), so a late V projection never blocks the next tile's scores and
  the exp stream keeps running.  The first three tiles' PVs wait until V0
  is projected (V0 slots into the PE idle at tile 2 while vt0 lands).
- Projections are emitted just-in-time at the last legal tile slot, where
  they can only block work that depends on them anyway.
- Two gate DMAs (tiny stores that read kt0 / q1) hold the load stream
  back so first wS+kt0, then q0+q1 get the wire alone: first exp ~20us.
- Output drains deferred past the input stream (c01 held in SBUF, drained
  during P4); c23 drains fine-grained over sync/scalar/gpsimd queues.
  Ghost matmuls keep the HAM clock at full speed through the drain.
- ACT does exps only; all PSUM->SBUF casts and merges are DVE.
"""

import math
from collections import deque
from contextlib import ExitStack

import numpy as np
import ml_dtypes

import concourse.bass as bass
import concourse.mybir as mybir
from concourse import bacc, tile
from concourse.bass_utils import run_bass_kernel_spmd

BF16 = mybir.dt.bfloat16
F32 = mybir.dt.float32
NPBF16 = ml_dtypes.bfloat16

B, S, D, DK, DV = 4, 4096, 1024, 128, 128
SK = 2048          # keys per core
NDCH = D // 128    # 8 contraction chunks
BLK = 512          # sk block
NBLK = SK // BLK   # 4 own blocks
SQC = 1024         # sq chunk processed per pass
NSQC = S // SQC    # 4
NT = BLK // 128    # 4 sk tiles per block
NG = S // 512      # 8 q column chunks

TRACE = False
TRACE_DIR = None
LAST_RESULT = None

Act = mybir.ActivationFunctionType


def build_nc():
    nc = bacc.Bacc(None, target_bir_lowering=False)

    qS = nc.declare_dram_parameter("qS", [NG, 128, NDCH * 512], BF16,
                                   isOutput=False)
    kS = nc.declare_dram_parameter("kS", [NBLK, 128, NDCH * 512], BF16,
                                   isOutput=False)
    vS = nc.declare_dram_parameter("vS", [NBLK, 128, NDCH * 512], BF16,
                                   isOutput=False)
    wS = nc.declare_dram_parameter("wS", [128, 3 * NDCH * 128], BF16,
                                   isOutput=False)
    bqp = nc.declare_dram_parameter("bq", [DK, 1], F32, isOutput=False)
    oT = nc.declare_dram_parameter("oT", [128, S], BF16, isOutput=True)
    accT = nc.declare_dram_parameter("accT", [128, S], BF16, isOutput=True)
    gateD = nc.dram_tensor("gateD", [128, 8], BF16)

    with tile.TileContext(nc) as tc:
        with (
            tc.tile_pool(name="const", bufs=1) as const,
            tc.tile_pool(name="wpool", bufs=1) as wpool,
            tc.tile_pool(name="persist", bufs=1) as persist,
            tc.tile_pool(name="kvstage", bufs=3) as kvstage,
            tc.tile_pool(name="qstage", bufs=3) as qpool,
            tc.tile_pool(name="ktile", bufs=4) as ktile_pool,
            tc.tile_pool(name="vtile", bufs=4) as vtile_pool,
            tc.tile_pool(name="attn", bufs=5) as attn_pool,
            tc.tile_pool(name="outp", bufs=2) as out_pool,
        ):
            dummy = const.tile([128, 512], BF16)
            nc.vector.memset(dummy[:], 0.125)
            bq_sb = const.tile([DK, 1], F32)
            nc.scalar.dma_start(bq_sb[:], bqp[:])

            QT_sb = persist.tile([128, S], BF16)           # [dk, sq]
            acc = persist.tile([128, S], BF16)             # exp-sum accumulator
            oA = persist.tile([128, S], BF16)              # blk01 O^T partial
            oM = persist.tile([128, SQC * 2], BF16)        # merged c01 (P4 drain)

            qtiles = [None] * NG
            kvt = [[None, None] for _ in range(NBLK)]

            def load_k(blk):
                kt = kvstage.tile([128, NDCH, 512], BF16, tag="kt")
                nc.sync.dma_start(kt[:], kS[blk].rearrange("p (c s) -> p c s",
                                                           c=NDCH))
                kvt[blk][0] = kt

            def load_v(blk):
                vt = kvstage.tile([128, NDCH, 512], BF16, tag="vt")
                nc.sync.dma_start(vt[:], vS[blk].rearrange("p (c s) -> p c s",
                                                           c=NDCH))
                kvt[blk][1] = vt

            def load_q(g):
                qt = qpool.tile([128, NDCH, 512], BF16, tag="q", name=f"q{g}")
                nc.sync.dma_start(
                    qt[:], qS[g].rearrange("p (c s) -> p c s", c=NDCH))
                qtiles[g] = qt

            # wire order with two gates: [wS kt0] | [q0 q1] | rest
            wsb = wpool.tile([128, 3, NDCH, 128], BF16)
            nc.sync.dma_start(wsb[:], wS.rearrange("p (w c m) -> p w c m",
                                                   w=3, c=NDCH))
            wk_sb = wsb[:, 0]
            wv_sb = wsb[:, 1]
            wq_sb = wsb[:, 2]
            load_k(0)
            nc.sync.dma_start(gateD[:], kvt[0][0][:, 0, 0:8])   # gate A (kt0)
            load_q(0)
            load_q(1)
            nc.sync.dma_start(gateD[:], qtiles[1][:, 0, 0:8])   # gate B (q1)
            load_v(0)
            load_k(1)
            load_v(1)
            load_q(2)
            load_q(3)
            load_k(2)
            load_v(2)
            load_k(3)
            load_v(3)
            load_q(4)
            load_q(5)
            load_q(6)
            load_q(7)

            # HAM warm-up: bridge preamble until kt0/wS land (~13us; the
            # first ~12 matmuls run at boot half-clock).
            with tc.tile_pool(name="psW", bufs=1, space="PSUM") as psW:
                wps = psW.tile([128, 512], F32)
                for i in range(46):
                    nc.tensor.matmul(wps[:, :256], dummy[:, :128],
                                     dummy[:, :256],
                                     start=(i == 0), stop=(i == 45))

            ksb = [None] * NBLK
            vsb = [None] * NBLK

            ctx = ExitStack()
            psSC = ctx.enter_context(
                tc.tile_pool(name="psSC", bufs=2, space="PSUM"))
            psOT = ctx.enter_context(
                tc.tile_pool(name="psOT", bufs=2, space="PSUM"))
            psA = ctx.enter_context(
                tc.tile_pool(name="psA", bufs=2, space="PSUM"))

            def proj_k(blk):
                kt = kvt[blk][0]
                kps = psA.tile([128, BLK], F32, tag="pj", name="pj")
                for c in range(NDCH):
                    nc.tensor.matmul(kps[:], wk_sb[:, c, :], kt[:, c, :],
                                     start=(c == 0), stop=(c == NDCH - 1))
                t = ktile_pool.tile([128, BLK], BF16, name="ksb")
                nc.vector.tensor_copy(t[:], kps[:])
                ksb[blk] = t

            def proj_v(blk):
                vt = kvt[blk][1]
                vps = psA.tile([128, BLK], F32, tag="pj", name="pj")
                for t in range(NT):
                    o = vps[:, t * DV:(t + 1) * DV]
                    for c in range(NDCH):
                        nc.tensor.matmul(o, vt[:, c, t * 128:(t + 1) * 128],
                                         wv_sb[:, c, :],
                                         start=(c == 0), stop=(c == NDCH - 1))
                tt = vtile_pool.tile([128, BLK], BF16, name="vsb")
                nc.vector.tensor_copy(tt[:], vps[:])
                vsb[blk] = tt

            def proj_q(g):
                qps = psA.tile([128, 512], F32, tag="pj", name="pj")
                for c in range(NDCH):
                    nc.tensor.matmul(qps[:], wq_sb[:, c, :], qtiles[g][:, c],
                                     start=(c == 0), stop=(c == NDCH - 1))
                nc.vector.tensor_scalar_add(QT_sb[:, g * 512:(g + 1) * 512],
                                            qps[:], bq_sb[:])

            # pre-stream: K0, Q0, Q1 (V0 is emitted at tile 2, when the PE
            # would otherwise idle on the psSC ring while vt0 lands)
            proj_k(0)
            proj_q(0)
            proj_q(1)

            # just-in-time projection slots: emitted after tile gidx's
            # scores/pops, right before the first tile that needs them
            fills = {
                2: [lambda: proj_v(0)],
                3: [lambda: proj_k(1)],
                4: [lambda: proj_v(1)],
                7: [lambda: proj_q(2), lambda: proj_q(3)],
                15: [lambda: proj_k(2)],
                16: [lambda: proj_v(2)],
                19: [lambda: proj_k(3)],
                20: [lambda: proj_v(3)],
                31: [lambda: proj_q(4), lambda: proj_q(5)],
                39: [lambda: proj_q(6), lambda: proj_q(7)],
            }

            pending = deque()   # deferred PV matmuls: (ot_g_list, blk, t, at, first, last)

            def emit_pv(ent):
                ot, blk, t, at, first, last = ent
                for g in range(2):
                    nc.tensor.matmul(
                        ot[g][:],
                        vsb[blk][:, t * 128:(t + 1) * 128],
                        at[:, g * 512:(g + 1) * 512],
                        start=first, stop=last)

            def flush_pv(depth):
                while len(pending) > depth:
                    emit_pv(pending.popleft())

            gidx = 0
            for half in (0, 1):          # chunks 01, then chunks 23
                for pb in (0, 1):        # block pair (0,1), then (2,3)
                    for ci in (0, 1):
                        sqc = 2 * half + ci
                        o0 = sqc * SQC
                        ot = [psOT.tile([128, 512], F32, tag="ot", name="ot")
                              for _ in range(2)]
                        for bi in (0, 1):
                            blk = 2 * pb + bi
                            for t in range(NT):
                                # scores
                                sc = psSC.tile([128, SQC], F32, tag="sc",
                                               name="sc")
                                for g in range(2):
                                    nc.tensor.matmul(
                                        sc[:, g * 512:(g + 1) * 512],
                                        ksb[blk][:, t * 128:(t + 1) * 128],
                                        QT_sb[:, o0 + g * 512:
                                              o0 + (g + 1) * 512],
                                        start=True, stop=True)
                                # exp (ACT's only job)
                                at = attn_pool.tile([128, SQC], BF16,
                                                    name="at")
                                if gidx == 63:
                                    # final tile: halves so PV/merge/drain
                                    # overlap the exp
                                    for g in range(2):
                                        nc.scalar.activation(
                                            at[:, g * 512:(g + 1) * 512],
                                            sc[:, g * 512:(g + 1) * 512],
                                            Act.Exp)
                                else:
                                    nc.scalar.activation(at[:], sc[:],
                                                         Act.Exp)
                                # deferred PVs (skew keeps scores flowing)
                                pending.append(
                                    (ot, blk, t, at,
                                     bi == 0 and t == 0,
                                     bi == 1 and t == NT - 1))
                                if gidx >= 3:
                                    flush_pv(1)
                                # JIT projections
                                for f in fills.get(gidx, ()):
                                    f()
                                # exp-sum accumulate (DVE)
                                aslice = acc[:, o0:o0 + SQC]
                                if pb == 0 and bi == 0 and t == 0:
                                    nc.vector.tensor_copy(aslice, at[:])
                                else:
                                    nc.vector.tensor_add(aslice, aslice,
                                                         at[:])
                                gidx += 1
                        flush_pv(0)
                        if pb == 0:
                            # spill blk01 partial to SBUF
                            for g in range(2):
                                nc.vector.tensor_copy(
                                    oA[:, o0 + g * 512:o0 + (g + 1) * 512],
                                    ot[g][:])
                        elif half == 0:
                            # merge c01; hold in SBUF, drain during P4
                            for g in range(2):
                                nc.vector.tensor_add(
                                    oM[:, o0 + g * 512:o0 + (g + 1) * 512],
                                    ot[g][:],
                                    oA[:, o0 + g * 512:o0 + (g + 1) * 512])
                        else:
                            # P4: merge + drain c23 now
                            last_pass = sqc == NSQC - 1
                            if last_pass:
                                nc.sync.dma_start(accT[:, o0:o0 + 512],
                                                  acc[:, o0:o0 + 512])
                                nc.scalar.dma_start(
                                    accT[:, o0 + 512:o0 + SQC],
                                    acc[:, o0 + 512:o0 + SQC])
                            else:
                                nc.sync.dma_start(accT[:, o0:o0 + SQC],
                                                  acc[:, o0:o0 + SQC])
                            for g in range(2):
                                osb = out_pool.tile([128, 512], BF16,
                                                    name="osb")
                                nc.vector.tensor_add(
                                    osb[:], ot[g][:],
                                    oA[:, o0 + g * 512:o0 + (g + 1) * 512])
                                o_q = (nc.gpsimd if (last_pass and g == 1)
                                       else nc.sync)
                                o_q.dma_start(
                                    oT[:, o0 + g * 512:o0 + (g + 1) * 512],
                                    osb[:])
                            if ci == 0:
                                # input stream is done: drain held c01
                                nc.sync.dma_start(oT[:, 0:SQC * 2],
                                                  oM[:, 0:SQC * 2])
                                nc.gpsimd.dma_start(accT[:, 0:SQC * 2],
                                                    acc[:, 0:SQC * 2])

            # ghost matmuls: hold the HAM clock through the final drain
            gps = psSC.tile([128, SQC], F32, tag="sc", name="sc")
            for i in range(24):
                nc.tensor.matmul(gps[:, :256], dummy[:, :128], dummy[:, :256],
                                 start=(i == 0), stop=(i == 23))
            ctx.close()

    nc.compile()
    return nc


def _swizzle(xT, nchunk):
    """[D, n*512] (partition-split along D) -> [n, 128, NDCH*512] so each
    chunk DMA reads one contiguous 8KB line per partition."""
    Dd, cols = xT.shape
    n = cols // 512
    x = xT.reshape(NDCH, 128, n, 512).transpose(2, 1, 0, 3)
    return np.ascontiguousarray(x.reshape(n, 128, NDCH * 512))


def _wswz(W):
    """[D, 128] -> [128, NDCH*128] per-partition contiguous."""
    return W.reshape(NDCH, 128, 128).transpose(1, 0, 2).reshape(128, NDCH * 128)


def kernel(q, k, v, Wq, bq, Wk, bk, Wv, bv):
    global LAST_RESULT
    q = np.asarray(q, np.float32)
    k = np.asarray(k, np.float32)
    v = np.asarray(v, np.float32)
    scale = 1.0 / math.sqrt(DK)

    wq_h = (np.asarray(Wq, np.float32) * scale).astype(NPBF16)
    wk_h = np.asarray(Wk, np.float32).astype(NPBF16)
    wv_h = np.asarray(Wv, np.float32).astype(NPBF16)
    wS_h = np.ascontiguousarray(
        np.concatenate([_wswz(wk_h), _wswz(wv_h), _wswz(wq_h)], axis=1))
    bq_h = (np.asarray(bq, np.float32) * scale).reshape(DK, 1)
    # bk shifts every score of a given query equally -> cancels in softmax;
    # bv passes straight through (attn rows sum to 1).

    qS_b = [_swizzle(q[b].T.astype(NPBF16), NG) for b in range(B)]

    in_maps = []
    for i in range(8):
        b, h = i // 2, i % 2
        kS_i = _swizzle(k[b, h * SK:(h + 1) * SK, :].T.astype(NPBF16), NBLK)
        vS_i = _swizzle(v[b, h * SK:(h + 1) * SK, :].T.astype(NPBF16), NBLK)
        in_maps.append({
            "qS": qS_b[b], "kS": kS_i, "vS": vS_i,
            "wS": wS_h, "bq": bq_h,
        })

    nc = build_nc()
    kwargs = {}
    if TRACE:
        kwargs = dict(trace=True, tmpdir=TRACE_DIR)
    res = run_bass_kernel_spmd(nc, in_maps, core_ids=list(range(8)), **kwargs)
    LAST_RESULT = res

    bv_f = np.asarray(bv, np.float32).reshape(1, DV)
    out = np.empty((B, S, DV), np.float32)
    for b in range(B):
        O = (res.results[2 * b]["oT"].astype(np.float32)
             + res.results[2 * b + 1]["oT"].astype(np.float32))    # [128, S]
        A = (res.results[2 * b]["accT"].astype(np.float32)
             + res.results[2 * b + 1]["accT"].astype(np.float32))  # [128, S]
        d = A.sum(axis=0)                                          # [S]
        out[b] = (O / d).T + bv_f
    return out


# revision 8
# speedup vs baseline: 1.1881x; 1.1881x over previous
"""Single-head attention (B=4, S=4096, D=1024, DK=DV=128) on 8 TRN2 NeuronCores.

Sharding: key-parallel -> core i handles batch i//2, KEY rows
[h*2048, (h+1)*2048) with h = i%2, for ALL 4096 queries; emits unnormalized
partial numerator O^T and exp-sum partials; host combines + normalizes.
Host prep: bf16 cast, fold 1/sqrt(DK) into Wq/bq, swizzle to 8KB lines;
bk dropped (cancels in softmax); bv added on host.

v3 schedule.  The kernel is bound by the ACT engine's exp stream (64 tiles x
~1.11us) and the input wire (~16.8MB at ~310 GB/s); everything else is
arranged so neither ever waits:

- 2x2 phase blocking: P1=blocks01 x chunks01, P2=blocks23 x chunks01,
  P3=blocks01 x chunks23, P4=blocks23 x chunks23.  O^T partials spill
  PSUM->SBUF bf16 after P1/P3 passes and merge in P2/P4.  This spreads DMA
  deadlines to match the single need-order wire stream (q2q3 by tile 8,
  kv2/3 by tiles 16-21, q4-7 by tile 32-40).
- PV skew: each tile's PV matmuls are emitted one tile late (a pending
  queue), so a late V projection never blocks the next tile's scores and
  the exp stream keeps running.  The first three tiles' PVs wait until V0
  is projected (V0 slots into the PE idle at tile 2 while vt0 lands).
- Projections are emitted just-in-time at the last legal tile slot, where
  they can only block work that depends on them anyway.
- Two gate DMAs (tiny stores that read kt0 / q1) hold the load stream
  back so first wS+kt0, then q0+q1 get the wire alone: first exp ~20us.
- Output drains deferred past the input stream (c01 held in SBUF, drained
  during P4); c23 drains fine-grained over sync/scalar/gpsimd queues.
  Ghost matmuls keep the HAM clock at full speed through the drain.
- ACT does exps only; all PSUM->SBUF casts and merges are DVE.
"""

import math
from collections import deque
from contextlib import ExitStack

import numpy as np
import ml_dtypes

import concourse.bass as bass
import concourse.mybir as mybir
from concourse import bacc, tile
from concourse.bass_utils import run_bass_kernel_spmd

BF16 = mybir.dt.bfloat16
F32 = mybir.dt.float32
NPBF16 = ml_dtypes.bfloat16

B, S, D, DK, DV = 4, 4096, 1024, 128, 128
SK = 2048          # keys per core
NDCH = D // 128    # 8 contraction chunks
BLK = 512          # sk block
NBLK = SK // BLK   # 4 own blocks
SQC = 1024         # sq chunk processed per pass
NSQC = S // SQC    # 4
NT = BLK // 128    # 4 sk tiles per block
NG = S // 512      # 8 q column chunks

TRACE = False
TRACE_DIR = None
LAST_RESULT = None

Act = mybir.ActivationFunctionType


def build_nc():
    nc = bacc.Bacc(None, target_bir_lowering=False)

    qS = nc.declare_dram_parameter("qS", [NG, 128, NDCH * 512], BF16,
                                   isOutput=False)
    kS = nc.declare_dram_parameter("kS", [NBLK, 128, NDCH * 512], BF16,
                                   isOutput=False)
    vS = nc.declare_dram_parameter("vS", [NBLK, 128, NDCH * 512], BF16,
                                   isOutput=False)
    wS = nc.declare_dram_parameter("wS", [128, 3 * NDCH * 128], BF16,
                                   isOutput=False)
    bqp = nc.declare_dram_parameter("bq", [DK, 1], F32, isOutput=False)
    oT = nc.declare_dram_parameter("oT", [128, S], BF16, isOutput=True)
    accT = nc.declare_dram_parameter("accT", [128, S], BF16, isOutput=True)
    gateD = nc.dram_tensor("gateD", [128, 8], BF16)

    with tile.TileContext(nc) as tc:
        with (
            tc.tile_pool(name="const", bufs=1) as const,
            tc.tile_pool(name="wpool", bufs=1) as wpool,
            tc.tile_pool(name="persist", bufs=1) as persist,
            tc.tile_pool(name="kvstage", bufs=3) as kvstage,
            tc.tile_pool(name="qstage", bufs=3) as qpool,
            tc.tile_pool(name="ktile", bufs=4) as ktile_pool,
            tc.tile_pool(name="vtile", bufs=4) as vtile_pool,
            tc.tile_pool(name="attn", bufs=5) as attn_pool,
            tc.tile_pool(name="outp", bufs=2) as out_pool,
        ):
            dummy = const.tile([128, 512], BF16)
            nc.vector.memset(dummy[:], 0.125)
            bq_sb = const.tile([DK, 1], F32)
            nc.scalar.dma_start(bq_sb[:], bqp[:])

            QT_sb = persist.tile([128, S], BF16)           # [dk, sq]
            acc = persist.tile([128, S], BF16)             # exp-sum accumulator
            oA = persist.tile([128, S], BF16)              # blk01 O^T partial
            oM = persist.tile([128, SQC * 2], BF16)        # merged c01 (P4 drain)

            qtiles = [None] * NG
            kvt = [[None, None] for _ in range(NBLK)]

            def load_k(blk, split=False):
                kt = kvstage.tile([128, NDCH, 512], BF16, tag="kt")
                ksrc = kS[blk].rearrange("p (c s) -> p c s", c=NDCH)
                if split:
                    h = NDCH // 2
                    nc.sync.dma_start(kt[:, :h], ksrc[:, :h])
                    nc.sync.dma_start(kt[:, h:], ksrc[:, h:])
                else:
                    nc.sync.dma_start(kt[:], ksrc)
                kvt[blk][0] = kt

            def load_v(blk, split=False):
                vt = kvstage.tile([128, NDCH, 512], BF16, tag="vt")
                vsrc = vS[blk].rearrange("p (c s) -> p c s", c=NDCH)
                if split:
                    h = NDCH // 2
                    nc.sync.dma_start(vt[:, :h], vsrc[:, :h])
                    nc.sync.dma_start(vt[:, h:], vsrc[:, h:])
                else:
                    nc.sync.dma_start(vt[:], vsrc)
                kvt[blk][1] = vt

            def load_q(g):
                qt = qpool.tile([128, NDCH, 512], BF16, tag="q", name=f"q{g}")
                nc.sync.dma_start(
                    qt[:], qS[g].rearrange("p (c s) -> p c s", c=NDCH))
                qtiles[g] = qt

            # wire order with two gates: [wS kt0] | [q0 q1] | rest
            wsb = wpool.tile([128, 3, NDCH, 128], BF16)
            nc.sync.dma_start(wsb[:], wS.rearrange("p (w c m) -> p w c m",
                                                   w=3, c=NDCH))
            wk_sb = wsb[:, 0]
            wv_sb = wsb[:, 1]
            wq_sb = wsb[:, 2]
            load_k(0, split=True)
            load_q(0)
            load_q(1)
            load_v(0, split=True)
            load_k(1)
            load_v(1)
            load_q(2)
            load_q(3)
            load_k(2)
            load_v(2)
            load_k(3)
            load_v(3)
            load_q(4)
            load_q(5)
            load_q(6)
            load_q(7)

            # HAM warm-up: bridge preamble until kt0/wS land (~13us; the
            # first ~12 matmuls run at boot half-clock).
            with tc.tile_pool(name="psW", bufs=1, space="PSUM") as psW:
                wps = psW.tile([128, 512], F32)
                for i in range(46):
                    nc.tensor.matmul(wps[:, :256], dummy[:, :128],
                                     dummy[:, :256],
                                     start=(i == 0), stop=(i == 45))

            ksb = [None] * NBLK
            vsb = [None] * NBLK

            ctx = ExitStack()
            psSC = ctx.enter_context(
                tc.tile_pool(name="psSC", bufs=2, space="PSUM"))
            psOT = ctx.enter_context(
                tc.tile_pool(name="psOT", bufs=2, space="PSUM"))
            psA = ctx.enter_context(
                tc.tile_pool(name="psA", bufs=2, space="PSUM"))

            def proj_k(blk):
                kt = kvt[blk][0]
                kps = psA.tile([128, BLK], F32, tag="pj", name="pj")
                for c in range(NDCH):
                    nc.tensor.matmul(kps[:], wk_sb[:, c, :], kt[:, c, :],
                                     start=(c == 0), stop=(c == NDCH - 1))
                t = ktile_pool.tile([128, BLK], BF16, name="ksb")
                nc.vector.tensor_copy(t[:], kps[:])
                ksb[blk] = t

            def proj_v(blk):
                vt = kvt[blk][1]
                vps = psA.tile([128, BLK], F32, tag="pj", name="pj")
                for t in range(NT):
                    o = vps[:, t * DV:(t + 1) * DV]
                    for c in range(NDCH):
                        nc.tensor.matmul(o, vt[:, c, t * 128:(t + 1) * 128],
                                         wv_sb[:, c, :],
                                         start=(c == 0), stop=(c == NDCH - 1))
                tt = vtile_pool.tile([128, BLK], BF16, name="vsb")
                nc.vector.tensor_copy(tt[:], vps[:])
                vsb[blk] = tt

            def proj_q(g):
                qps = psA.tile([128, 512], F32, tag="pj", name="pj")
                for c in range(NDCH):
                    nc.tensor.matmul(qps[:], wq_sb[:, c, :], qtiles[g][:, c],
                                     start=(c == 0), stop=(c == NDCH - 1))
                nc.vector.tensor_scalar_add(QT_sb[:, g * 512:(g + 1) * 512],
                                            qps[:], bq_sb[:])

            # pre-stream: K0, Q0, Q1 (V0 is emitted at tile 2, when the PE
            # would otherwise idle on the psSC ring while vt0 lands)
            proj_k(0)
            proj_q(0)
            proj_q(1)

            # just-in-time projection slots: emitted after tile gidx's
            # scores/pops, right before the first tile that needs them
            fills = {
                2: [lambda: proj_v(0)],
                3: [lambda: proj_k(1)],
                4: [lambda: proj_v(1)],
                7: [lambda: proj_q(2), lambda: proj_q(3)],
                15: [lambda: proj_k(2)],
                16: [lambda: proj_v(2)],
                19: [lambda: proj_k(3)],
                20: [lambda: proj_v(3)],
                31: [lambda: proj_q(4), lambda: proj_q(5)],
                39: [lambda: proj_q(6), lambda: proj_q(7)],
            }

            pending = deque()   # deferred PV matmuls: (ot_g_list, blk, t, at, first, last)

            def emit_pv(ent):
                ot, blk, t, at, first, last = ent
                for g in range(2):
                    nc.tensor.matmul(
                        ot[g][:],
                        vsb[blk][:, t * 128:(t + 1) * 128],
                        at[:, g * 512:(g + 1) * 512],
                        start=first, stop=last)

            def flush_pv(depth):
                while len(pending) > depth:
                    emit_pv(pending.popleft())

            gidx = 0
            for half in (0, 1):          # chunks 01, then chunks 23
                for pb in (0, 1):        # block pair (0,1), then (2,3)
                    for ci in (0, 1):
                        sqc = 2 * half + ci
                        o0 = sqc * SQC
                        ot = [psOT.tile([128, 512], F32, tag="ot", name="ot")
                              for _ in range(2)]
                        for bi in (0, 1):
                            blk = 2 * pb + bi
                            for t in range(NT):
                                # scores
                                sc = psSC.tile([128, SQC], F32, tag="sc",
                                               name="sc")
                                for g in range(2):
                                    nc.tensor.matmul(
                                        sc[:, g * 512:(g + 1) * 512],
                                        ksb[blk][:, t * 128:(t + 1) * 128],
                                        QT_sb[:, o0 + g * 512:
                                              o0 + (g + 1) * 512],
                                        start=True, stop=True)
                                # exp (ACT's only job)
                                at = attn_pool.tile([128, SQC], BF16,
                                                    name="at")
                                if gidx == 63:
                                    # final tile: halves so PV/merge/drain
                                    # overlap the exp
                                    for g in range(2):
                                        nc.scalar.activation(
                                            at[:, g * 512:(g + 1) * 512],
                                            sc[:, g * 512:(g + 1) * 512],
                                            Act.Exp)
                                else:
                                    nc.scalar.activation(at[:], sc[:],
                                                         Act.Exp)
                                # deferred PVs (skew keeps scores flowing)
                                pending.append(
                                    (ot, blk, t, at,
                                     bi == 0 and t == 0,
                                     bi == 1 and t == NT - 1))
                                if gidx >= 3:
                                    flush_pv(1)
                                # JIT projections
                                for f in fills.get(gidx, ()):
                                    f()
                                # exp-sum accumulate (DVE)
                                aslice = acc[:, o0:o0 + SQC]
                                if pb == 0 and bi == 0 and t == 0:
                                    nc.vector.tensor_copy(aslice, at[:])
                                else:
                                    nc.vector.tensor_add(aslice, aslice,
                                                         at[:])
                                gidx += 1
                        flush_pv(0)
                        if pb == 0:
                            # spill blk01 partial to SBUF
                            for g in range(2):
                                nc.vector.tensor_copy(
                                    oA[:, o0 + g * 512:o0 + (g + 1) * 512],
                                    ot[g][:])
                        elif half == 0:
                            # merge c01; hold in SBUF, drain during P4
                            for g in range(2):
                                nc.vector.tensor_add(
                                    oM[:, o0 + g * 512:o0 + (g + 1) * 512],
                                    ot[g][:],
                                    oA[:, o0 + g * 512:o0 + (g + 1) * 512])
                        else:
                            # P4: merge + drain c23 now
                            last_pass = sqc == NSQC - 1
                            if last_pass:
                                nc.sync.dma_start(accT[:, o0:o0 + 512],
                                                  acc[:, o0:o0 + 512])
                                nc.scalar.dma_start(
                                    accT[:, o0 + 512:o0 + SQC],
                                    acc[:, o0 + 512:o0 + SQC])
                            else:
                                nc.sync.dma_start(accT[:, o0:o0 + SQC],
                                                  acc[:, o0:o0 + SQC])
                            for g in range(2):
                                osb = out_pool.tile([128, 512], BF16,
                                                    name="osb")
                                nc.vector.tensor_add(
                                    osb[:], ot[g][:],
                                    oA[:, o0 + g * 512:o0 + (g + 1) * 512])
                                o_q = (nc.gpsimd if (last_pass and g == 1)
                                       else nc.sync)
                                o_q.dma_start(
                                    oT[:, o0 + g * 512:o0 + (g + 1) * 512],
                                    osb[:])
                            if ci == 0:
                                # input stream is done: drain held c01
                                nc.sync.dma_start(oT[:, 0:SQC * 2],
                                                  oM[:, 0:SQC * 2])
                                nc.gpsimd.dma_start(accT[:, 0:SQC * 2],
                                                    acc[:, 0:SQC * 2])

            # ghost matmuls: hold the HAM clock through the final drain
            gps = psSC.tile([128, SQC], F32, tag="sc", name="sc")
            for i in range(24):
                nc.tensor.matmul(gps[:, :256], dummy[:, :128], dummy[:, :256],
                                 start=(i == 0), stop=(i == 23))
            ctx.close()

    nc.compile()
    return nc


def _swizzle(xT, nchunk):
    """[D, n*512] (partition-split along D) -> [n, 128, NDCH*512] so each
    chunk DMA reads one contiguous 8KB line per partition."""
    Dd, cols = xT.shape
    n = cols // 512
    x = xT.reshape(NDCH, 128, n, 512).transpose(2, 1, 0, 3)
    return np.ascontiguousarray(x.reshape(n, 128, NDCH * 512))


def _wswz(W):
    """[D, 128] -> [128, NDCH*128] per-partition contiguous."""
    return W.reshape(NDCH, 128, 128).transpose(1, 0, 2).reshape(128, NDCH * 128)


def kernel(q, k, v, Wq, bq, Wk, bk, Wv, bv):
    global LAST_RESULT
    q = np.asarray(q, np.float32)
    k = np.asarray(k, np.float32)
    v = np.asarray(v, np.float32)
    scale = 1.0 / math.sqrt(DK)

    wq_h = (np.asarray(Wq, np.float32) * scale).astype(NPBF16)
    wk_h = np.asarray(Wk, np.float32).astype(NPBF16)
    wv_h = np.asarray(Wv, np.float32).astype(NPBF16)
    wS_h = np.ascontiguousarray(
        np.concatenate([_wswz(wk_h), _wswz(wv_h), _wswz(wq_h)], axis=1))
    bq_h = (np.asarray(bq, np.float32) * scale).reshape(DK, 1)
    # bk shifts every score of a given query equally -> cancels in softmax;
    # bv passes straight through (attn rows sum to 1).

    qS_b = [_swizzle(q[b].T.astype(NPBF16), NG) for b in range(B)]

    in_maps = []
    for i in range(8):
        b, h = i // 2, i % 2
        kS_i = _swizzle(k[b, h * SK:(h + 1) * SK, :].T.astype(NPBF16), NBLK)
        vS_i = _swizzle(v[b, h * SK:(h + 1) * SK, :].T.astype(NPBF16), NBLK)
        in_maps.append({
            "qS": qS_b[b], "kS": kS_i, "vS": vS_i,
            "wS": wS_h, "bq": bq_h,
        })

    nc = build_nc()
    kwargs = {}
    if TRACE:
        kwargs = dict(trace=True, tmpdir=TRACE_DIR)
    res = run_bass_kernel_spmd(nc, in_maps, core_ids=list(range(8)), **kwargs)
    LAST_RESULT = res

    bv_f = np.asarray(bv, np.float32).reshape(1, DV)
    out = np.empty((B, S, DV), np.float32)
    for b in range(B):
        O = (res.results[2 * b]["oT"].astype(np.float32)
             + res.results[2 * b + 1]["oT"].astype(np.float32))    # [128, S]
        A = (res.results[2 * b]["accT"].astype(np.float32)
             + res.results[2 * b + 1]["accT"].astype(np.float32))  # [128, S]
        d = A.sum(axis=0)                                          # [S]
        out[b] = (O / d).T + bv_f
    return out
